# revision 45
# baseline (speedup 1.0000x reference)
"""Location-sensitive attention Bass kernel for Trainium2 (8 NeuronCores).

Strategy: pure data parallel over the batch dim (64 batches -> 8 per core).
The computation is window-sparse: each batch row only touches a 128-wide
window of the 4096-long time axis, selected by `window_start`. All
data-dependent addressing is done with GPSIMD indirect DMAs / dma_gather
whose index tensors are computed on the host from `window_start` and fed as
inputs, so a single SPMD program serves all 8 cores.

Per-core device program:
  1. Two packed small-input loads (indices + parameters, bitcast views)
     so gathers can start early; enc windows via dma_gather (one 512B
     descriptor per (w, b), landing as [w, b, h] for the PE).
  2. cumulative_alignment + tokens_mask in a [128, 256] chunked layout
     (8 rows x 16 chunks -> 128 partitions), mask applied on DVE.
  3. Bulk-write align_full = zeros and cum_new = masked cum.
  4. Padded "loc" rows ([init x15 | masked cum | zeros]) in DRAM scratch,
     window-gathered to position 0, then the conv Toeplitz operand
     X[k, (b,w)] = loc[b, ws_b + k + w] loaded via an overlapping AP.
  5. qp = Wq @ query (PE, K=1024), conv as a K=31 matmul, tanh(+bias) on
     ACT, score via per-batch M=1 matmuls into a [1, 1024] PSUM row
     (PE outputs must start at partition 0), folded to [8, 128] through
     a DRAM bounce (SBUF->SBUF reshape DMAs scramble partitions on HW),
     masked softmax on DVE/ACT.
  6. context via per-batch M=1 matmuls against the gathered enc windows.
  7. Indirect-scatter align_w into align_full and scatter-with-CCE-add
     into cum_new; ws_new via an iota/min argmax trick.
"""

import os
import sys
import types

import numpy as np

for _p in ("/root/.axon_site/_ro/trn_rl_repo", "/opt/trn_rl_repo"):
    if os.path.isdir(_p) and _p not in sys.path:
        sys.path.append(_p)

T, B, H, Q = 4096, 64, 128, 1024
K = 31
PAD = 15
W = 128
NCORES = 8
BC = B // NCORES          # batches per core
LP = 4224                 # loc_pad row length (15 + 4096 + zero tail)
CF = 256                  # chunk free-dim length; 8 rows * 16 chunks = 128
S8C = 144                 # packed [BC, S8C] i32 small-input tensor
S128C = 227               # packed [128, S128C] i32 small-input tensor

_CACHE = {}
last_results = None       # BassKernelResults of the most recent run


def _install_trace_hook():
    """Register the axon NTFF profiling hook so trace=True works."""
    try:
        import antenv.axon_hooks  # noqa: F401
        return
    except ImportError:
        pass
    mod = types.ModuleType("antenv.axon_hooks")
    _h = [None]
    mod.set_axon_ntff_profile_hook = lambda h: _h.__setitem__(0, h)
    mod.get_axon_ntff_profile_hook = lambda: _h[0]
    sys.modules["antenv.axon_hooks"] = mod
    try:
        from trn_agent_boot.trn_boot import _ntff_profile_via_ctypes

        hook = _ntff_profile_via_ctypes("/opt/axon/libaxon_pjrt.so")
        if hook is not None:
            mod.set_axon_ntff_profile_hook(hook)
    except Exception:
        pass


def _build_program():
    import concourse.bacc as bacc
    import concourse.bass as bass
    import concourse.mybir as mybir
    import concourse.tile as tile

    dt = mybir.dt
    f32, i32, u8, i16 = dt.float32, dt.int32, dt.uint8, dt.int16
    AX = mybir.AxisListType.X
    OP = mybir.AluOpType
    ACT = mybir.ActivationFunctionType

    nc = bacc.Bacc("TRN2", target_bir_lowering=False, debug=False,
                   num_devices=NCORES)

    enc = nc.dram_tensor("enc", [BC, T, H], f32, kind="ExternalInput")
    mask = nc.dram_tensor("mask", [BC, T], u8, kind="ExternalInput")
    cum = nc.dram_tensor("cum", [BC, T], f32, kind="ExternalInput")
    wq = nc.dram_tensor("wq", [H, Q], f32, kind="ExternalInput")
    sm8 = nc.dram_tensor("sm8", [BC, S8C], i32, kind="ExternalInput")
    sm128 = nc.dram_tensor("sm128", [128, S128C], i32, kind="ExternalInput")

    ctx_o = nc.dram_tensor("context", [BC, H], f32, kind="ExternalOutput")
    cumn_o = nc.dram_tensor("cum_new", [BC, T], f32, kind="ExternalOutput")
    alif_o = nc.dram_tensor("align_full", [BC, T], f32, kind="ExternalOutput")
    wsn_o = nc.dram_tensor("ws_new", [BC, 1], i32, kind="ExternalOutput")

    def chunked(ap):
        return ap.rearrange("b (c f) -> b c f", f=CF)

    with tile.TileContext(nc) as tc:
        with (
            tc.tile_pool(name="sb", bufs=1) as sb,
            tc.tile_pool(name="ps", bufs=1, space="PSUM") as ps,
            tc.tile_pool(name="dr", bufs=1, space="DRAM") as dr,
        ):
            # ---- warm the ACT function table off the critical path
            warm = sb.tile([1, 2], f32)
            nc.vector.memset(warm[:], 0.0)
            nc.scalar.activation(out=warm[:], in_=warm[:], func=ACT.Tanh)
            nc.scalar.activation(out=warm[:], in_=warm[:], func=ACT.Exp)

            # ---- two packed small-input loads; bitcast slice views
            s8 = sb.tile([BC, S8C], i32)
            nc.sync.dma_start(s8[:], sm8[:])
            s128 = sb.tile([128, S128C], i32)
            nc.sync.dma_start(s128[:], sm128[:])
            lidx_v = s8[:, 0:1]
            widx_v = s8[:, 1:2]
            ntok_v = s8[:, 2:3]
            wsf_v = s8[:, 3:4].bitcast(f32)
            initv_v = s8[:, 4:5].bitcast(f32)
            id8_v = s8[:, 8:16].bitcast(f32)
            iot_v = s8[:, 16:144].bitcast(f32)
            gidx_v = s128[:, 0:32].bitcast(i16)
            q_v = s128[:, 32:96].bitcast(f32)
            v_v = s128[:, 96:97].bitcast(f32)
            bhb_v = s128[:, 97:98].bitcast(f32)
            cwt_v = s128[0:31, 98:226].bitcast(f32)
            ones_v = s128[:, 226:227].bitcast(f32)



            # ---- bulk chunked cum/mask processing
            cum_sb = sb.tile([128, CF], f32)
            nc.sync.dma_start(cum_sb[:], chunked(cum[:]))
            mask_sb = sb.tile([128, CF], u8)
            nc.sync.dma_start(mask_sb[:], chunked(mask[:]))
            wq_sb = sb.tile([H, Q], f32)
            nc.sync.dma_start(wq_sb[:], wq[:])
            maskf = sb.tile([128, CF], f32)
            nc.vector.tensor_copy(maskf[:], mask_sb[:])
            cmask = sb.tile([128, CF], f32)
            nc.vector.tensor_mul(cmask[:], cum_sb[:], maskf[:])
            zc = sb.tile([128, CF], f32)
            nc.vector.memset(zc[:], 0.0)

            # bulk outputs (windows patched by indirect scatters below)
            nc.scalar.dma_start(chunked(alif_o[:]), zc[:])
            nc.scalar.dma_start(chunked(cumn_o[:]), cmask[:])

            # ---- loc_pad = [initial x PAD | masked cum | zeros]
            loc_pad = dr.tile([BC, LP], f32)
            nc.scalar.dma_start(chunked(loc_pad[:, PAD:PAD + T]), cmask[:])
            init15 = sb.tile([BC, PAD], f32)
            nc.vector.tensor_copy(init15[:], initv_v.to_broadcast([BC, PAD]))
            nc.scalar.dma_start(loc_pad[:, 0:PAD], init15[:])
            zt = sb.tile([BC, 128], f32)
            nc.vector.memset(zt[:], 0.0)
            nc.scalar.dma_start(loc_pad[:, PAD + T:LP], zt[:, :LP - PAD - T])

            # per-batch padded loc windows (one idx per partition, HW indirect
            # DMA streams the full dest row per index)
            locw = sb.tile([BC, 160], f32)
            locw_i = nc.gpsimd.indirect_dma_start(
                out=locw[:], out_offset=None, in_=loc_pad[:],
                in_offset=bass.IndirectOffsetOnAxis(ap=lidx_v, axis=1))
            locw_d = dr.tile([BC, 160], f32)
            nc.scalar.dma_start(locw_d[:], locw[:])
            # Toeplitz X[k, (b, w)] = locw[b, k + w] via overlapping DRAM AP
            X = sb.tile([K, BC * W], f32)
            x_src = bass.AP(locw_d[:].tensor, 0,
                            [[1, K], [160, BC], [1, W]])
            x_l = nc.gpsimd.dma_start(X[:], x_src)
            # window mask gather (one idx per partition)
            mw_u8 = sb.tile([BC, W], u8)
            nc.gpsimd.indirect_dma_start(
                out=mw_u8[:], out_offset=None, in_=mask[:],
                in_offset=bass.IndirectOffsetOnAxis(ap=widx_v, axis=1))

            # ---- enc windows via dma_gather: idx i -> dst partition i%128
            # (=w), block i//128 (=b); row i reads enc[(b t), h] at t=ws_b+w.
            # The real gather costs ~9us of Q7 time (the cost model thinks
            # ~1us) and its result is only needed by the late context
            # matmuls, so force it AFTER the locw gather on the Pool queue.
            enc_sb = sb.tile([W, BC, H], f32)
            enc_g = nc.gpsimd.dma_gather(
                enc_sb[:], enc[:].rearrange("b t h -> (b t) h"), gidx_v,
                BC * W, BC * W, H)
            tile.add_dep_helper(enc_g.ins, x_l.ins, sync=False,
                                reason="run the slow enc gather after X")

            # ---- qp = Wq @ query (chunked over K=1024) + (bq + conv_b)
            qp_ps = ps.tile([H, BC], f32)
            for c in range(8):
                nc.tensor.matmul(
                    out=qp_ps[:], lhsT=wq_sb[:, c * 128:(c + 1) * 128],
                    rhs=q_v[:, c * BC:(c + 1) * BC],
                    start=(c == 0), stop=(c == 7))
            qpb = sb.tile([H, BC], f32)
            nc.vector.tensor_scalar(out=qpb[:], in0=qp_ps[:],
                                    scalar1=bhb_v, scalar2=None,
                                    op0=OP.add)

            # ---- conv as K=31 matmul over the Toeplitz gather
            feats_ps = ps.tile([H, BC * W], f32)
            nc.tensor.matmul(out=feats_ps[:, 0:512], lhsT=cwt_v,
                             rhs=X[:, 0:512], start=True, stop=True)
            nc.tensor.matmul(out=feats_ps[:, 512:1024], lhsT=cwt_v,
                             rhs=X[:, 512:1024], start=True, stop=True)

            # ---- tanh(feats + qp_b + bias)
            th = sb.tile([H, BC * W], f32)
            for b in range(BC):
                nc.scalar.activation(out=th[:, b * W:(b + 1) * W],
                                     in_=feats_ps[:, b * W:(b + 1) * W],
                                     func=ACT.Tanh, bias=qpb[:, b:b + 1],
                                     scale=1.0)

            # ---- score[b, w] = v . tanh_feats (per-batch M=1 matmuls into
            # a [1, BC*W] PSUM row; PE outputs must start at partition 0)
            score_ps = ps.tile([1, BC * W], f32)
            for b in range(BC):
                nc.tensor.matmul(out=score_ps[:, b * W:(b + 1) * W],
                                 lhsT=v_v,
                                 rhs=th[:, b * W:(b + 1) * W],
                                 start=True, stop=True)
            score_row = sb.tile([1, BC * W], f32)
            nc.vector.tensor_copy(score_row[:, 0:512], score_ps[:, 0:512])
            nc.scalar.copy(out=score_row[:, 512:1024],
                           in_=score_ps[:, 512:1024])
            # SBUF->SBUF reshape DMAs scramble partitions on HW; bounce
            # through DRAM to fold [1, BC*W] into [BC, W].
            score_d = dr.tile([BC, W], f32)
            nc.scalar.dma_start(
                score_d[:].rearrange("b w -> (b w)")[None, :], score_row[:])
            score8 = sb.tile([BC, W], f32)
            nc.sync.dma_start(score8[:], score_d[:])

            # ---- mask + softmax over the window
            mwf = sb.tile([BC, W], f32)
            nc.vector.tensor_copy(mwf[:], mw_u8[:])
            pen = sb.tile([BC, W], f32)
            nc.vector.tensor_scalar(out=pen[:], in0=mwf[:], scalar1=1e30,
                                    scalar2=-1e30, op0=OP.mult, op1=OP.add)
            score_sb = sb.tile([BC, W], f32)
            nc.vector.tensor_add(score_sb[:], score8[:], pen[:])
            mx = sb.tile([BC, 1], f32)
            nc.vector.reduce_max(mx[:], score_sb[:], axis=AX)
            negmx = sb.tile([BC, 1], f32)
            nc.vector.tensor_scalar_mul(negmx[:], mx[:], -1.0)
            ex = sb.tile([BC, W], f32)
            nc.scalar.activation(out=ex[:], in_=score_sb[:], func=ACT.Exp,
                                 bias=negmx[:], scale=1.0)
            sm = sb.tile([BC, 1], f32)
            nc.vector.reduce_sum(sm[:], ex[:], axis=AX)
            rs = sb.tile([BC, 1], f32)
            nc.vector.reciprocal(rs[:], sm[:])
            aw = sb.tile([BC, W], f32)
            nc.vector.tensor_scalar(out=aw[:], in0=ex[:], scalar1=rs[:],
                                    scalar2=None, op0=OP.mult)

            # ---- transpose align_w -> [W, BC] for the context matmuls
            awT_ps = ps.tile([W, BC], f32)
            nc.tensor.transpose(out=awT_ps[:], in_=aw[:], identity=id8_v)
            awT = sb.tile([W, BC], f32)
            nc.vector.tensor_copy(awT[:], awT_ps[:])

            # ---- context[b, h] = sum_w aw[b, w] * enc[w, b, h]
            # ([1, BC*H] PSUM row == row-major [BC, H] when flattened)
            ctx_ps = ps.tile([1, BC * H], f32)
            for b in range(BC):
                nc.tensor.matmul(out=ctx_ps[:, b * H:(b + 1) * H],
                                 lhsT=awT[:, b:b + 1],
                                 rhs=enc_sb[:, b, :],
                                 start=True, stop=True)
            ctx_sb = sb.tile([1, BC * H], f32)
            nc.vector.tensor_copy(ctx_sb[:, 0:512], ctx_ps[:, 0:512])
            nc.scalar.copy(out=ctx_sb[:, 512:1024], in_=ctx_ps[:, 512:1024])
            nc.sync.dma_start(
                ctx_o[:].rearrange("b h -> (b h)")[None, :], ctx_sb[:])

            # ---- scatter windows into the bulk outputs
            nc.gpsimd.indirect_dma_start(
                out=alif_o[:],
                out_offset=bass.IndirectOffsetOnAxis(ap=widx_v, axis=1),
                in_=aw[:], in_offset=None)
            nc.gpsimd.indirect_dma_start(
                out=cumn_o[:],
                out_offset=bass.IndirectOffsetOnAxis(ap=widx_v, axis=1),
                in_=aw[:], in_offset=None, compute_op=OP.add)

            # ---- ws_new = clip(ws + argmax(aw) - W//2, 0, ntok - W)
            amax = sb.tile([BC, 1], f32)
            nc.vector.reduce_max(amax[:], aw[:], axis=AX)
            eqm = sb.tile([BC, W], f32)
            nc.vector.tensor_scalar(out=eqm[:], in0=aw[:], scalar1=amax[:],
                                    scalar2=None, op0=OP.is_equal)
            t1 = sb.tile([BC, W], f32)
            nc.vector.tensor_scalar(out=t1[:], in0=eqm[:], scalar1=-4096.0,
                                    scalar2=4096.0, op0=OP.mult, op1=OP.add)
            t2 = sb.tile([BC, W], f32)
            nc.vector.tensor_mul(t2[:], iot_v, eqm[:])
            idxm = sb.tile([BC, W], f32)
            nc.vector.tensor_add(idxm[:], t1[:], t2[:])
            fidx = sb.tile([BC, 1], f32)
            nc.vector.tensor_reduce(fidx[:], idxm[:], axis=AX, op=OP.min)
            wsn = sb.tile([BC, 1], f32)
            nc.vector.tensor_scalar(out=wsn[:], in0=fidx[:],
                                    scalar1=wsf_v, scalar2=float(-(W // 2)),
                                    op0=OP.add, op1=OP.add)
            ntf = sb.tile([BC, 1], f32)
            nc.vector.tensor_copy(ntf[:], ntok_v)
            lim = sb.tile([BC, 1], f32)
            nc.vector.tensor_scalar_add(lim[:], ntf[:], float(-W))
            wsn2 = sb.tile([BC, 1], f32)
            nc.vector.tensor_tensor(wsn2[:], wsn[:], lim[:], op=OP.min)
            wsn3 = sb.tile([BC, 1], f32)
            nc.vector.tensor_scalar_max(wsn3[:], wsn2[:], 0.0)
            wsn_i = sb.tile([BC, 1], i32)
            nc.vector.tensor_copy(wsn_i[:], wsn3[:])
            nc.sync.dma_start(wsn_o[:], wsn_i[:])

    nc.compile()
    return nc


def _prep_core_inputs(core, encoded_tokens, tokens_mask, num_tokens, query,
                      cumulative_alignment, initial_cumulative_alignment,
                      window_start, wq_l, cwt_l, v_col, bhb_col):
    bs = slice(core * BC, (core + 1) * BC)
    ws = window_start[bs].astype(np.int64)

    qc = query[0, bs, :]                       # [BC, Q]
    q_l = np.ascontiguousarray(
        qc.T.reshape(8, 128, BC).transpose(1, 0, 2).reshape(128, 8 * BC))

    # dma_gather indices: unwrapped L[i] (i = b*W + w) = row b*T + ws_b + w
    # of enc viewed as [(b t), h]; wrapped layout [p, s] = L[s*16 + p%16],
    # replicated across the 8 GPSIMD cores (partition groups of 16).
    w_ar = np.arange(W, dtype=np.int64)
    L = (np.arange(BC, dtype=np.int64)[:, None] * T
         + ws[:, None] + w_ar[None, :]).reshape(-1)             # [BC*W]
    wrapped = L.reshape((BC * W) // 16, 16).T                   # [16, n/16]
    gidx = np.tile(wrapped, (8, 1)).astype(np.int16)            # [128, n/16]

    # packed [BC, S8C] i32 tensor
    s8 = np.zeros((BC, S8C), dtype=np.int32)
    s8[:, 0] = (np.arange(BC) * LP + ws).astype(np.int32)       # lidx
    s8[:, 1] = (np.arange(BC) * T + ws).astype(np.int32)        # widx
    s8[:, 2] = num_tokens[bs].astype(np.int32)                  # ntok
    s8[:, 3] = ws.astype(np.float32).view(np.int32)             # wsf
    s8[:, 4] = initial_cumulative_alignment[bs, 0].astype(
        np.float32).view(np.int32)                              # initv
    s8[:, 8:16] = np.eye(BC, dtype=np.float32).view(np.int32)   # id8
    s8[:, 16:144] = np.tile(np.arange(W, dtype=np.float32),
                            (BC, 1)).view(np.int32)             # iota

    # packed [128, S128C] i32 tensor
    s128 = np.zeros((128, S128C), dtype=np.int32)
    s128[:, 0:32] = gidx.view(np.int32)
    s128[:, 32:96] = q_l.astype(np.float32).view(np.int32)
    s128[:, 96] = v_col[:, 0].view(np.int32)
    s128[:, 97] = bhb_col[:, 0].view(np.int32)
    s128[0:31, 98:226] = cwt_l.view(np.int32)
    s128[:, 226] = np.float32(1.0).view(np.int32)

    return {
        "enc": np.ascontiguousarray(encoded_tokens[:, bs, :].transpose(1, 0, 2)),
        "mask": np.ascontiguousarray(tokens_mask[bs, :]).astype(np.uint8),
        "cum": np.ascontiguousarray(cumulative_alignment[bs, :]),
        "wq": wq_l,
        "sm8": s8,
        "sm128": s128,
    }


def make_in_maps(encoded_tokens, tokens_mask, num_tokens, query,
                 cumulative_alignment, initial_cumulative_alignment,
                 window_start, Wq, bq, conv_w, conv_b, v):
    encoded_tokens = np.asarray(encoded_tokens, dtype=np.float32)
    tokens_mask = np.asarray(tokens_mask)
    num_tokens = np.asarray(num_tokens, dtype=np.int32)
    query = np.asarray(query, dtype=np.float32)
    cumulative_alignment = np.asarray(cumulative_alignment, dtype=np.float32)
    initial_cumulative_alignment = np.asarray(
        initial_cumulative_alignment, dtype=np.float32)
    window_start = np.asarray(window_start, dtype=np.int32)
    Wq = np.asarray(Wq, dtype=np.float32)
    bq = np.asarray(bq, dtype=np.float32)
    conv_w = np.asarray(conv_w, dtype=np.float32)
    conv_b = np.asarray(conv_b, dtype=np.float32)
    v = np.asarray(v, dtype=np.float32)

    wq_l = np.ascontiguousarray(
        Wq.T.reshape(8, 128, H).transpose(1, 0, 2).reshape(128, Q))
    cwt_l = np.ascontiguousarray(conv_w[:, 0, :].T)     # [K, H]
    v_col = np.ascontiguousarray(v.reshape(H, 1))
    bhb_col = np.ascontiguousarray((bq + conv_b).reshape(H, 1))

    return [
        _prep_core_inputs(c, encoded_tokens, tokens_mask, num_tokens, query,
                          cumulative_alignment, initial_cumulative_alignment,
                          window_start, wq_l, cwt_l, v_col, bhb_col)
        for c in range(NCORES)
    ]


def kernel(**inputs):
    global last_results
    trace = bool(os.environ.get("KERNEL_TRACE"))
    if trace:
        _install_trace_hook()

    from concourse.bass_utils import run_bass_kernel_spmd

    if "nc" not in _CACHE:
        _CACHE["nc"] = _build_program()
    nc = _CACHE["nc"]

    in_maps = make_in_maps(**inputs)
    res = run_bass_kernel_spmd(nc, in_maps, core_ids=list(range(NCORES)),
                               trace=trace)
    last_results = res

    context = np.concatenate([r["context"] for r in res.results], axis=0)
    cum_new = np.concatenate([r["cum_new"] for r in res.results], axis=0)
    align_full = np.concatenate([r["align_full"] for r in res.results], axis=0)
    ws_new = np.concatenate(
        [r["ws_new"].reshape(BC) for r in res.results], axis=0)
    return context, cum_new, align_full, ws_new


# revision 46
# speedup vs baseline: 1.0802x; 1.0802x over previous
"""Location-sensitive attention Bass kernel for Trainium2 (8 NeuronCores).

Strategy: pure data parallel over the batch dim (64 batches -> 8 per core).
The computation is window-sparse: each batch row only touches a 128-wide
window of the 4096-long time axis, selected by `window_start`. All
data-dependent addressing is done with GPSIMD indirect DMAs / dma_gather
whose index tensors are computed on the host from `window_start` and fed as
inputs, so a single SPMD program serves all 8 cores.

Per-core device program:
  1. Two packed small-input loads (indices + parameters, bitcast views)
     so gathers can start early; enc windows via dma_gather (one 512B
     descriptor per (w, b), landing as [w, b, h] for the PE).
  2. cumulative_alignment + tokens_mask in a [128, 256] chunked layout
     (8 rows x 16 chunks -> 128 partitions), mask applied on DVE.
  3. Bulk-write align_full = zeros and cum_new = masked cum.
  4. Padded "loc" rows ([init x15 | masked cum | zeros]) in DRAM scratch,
     window-gathered to position 0, then the conv Toeplitz operand
     X[k, (b,w)] = loc[b, ws_b + k + w] loaded via an overlapping AP.
  5. qp = Wq @ query (PE, K=1024), conv as a K=31 matmul, tanh(+bias) on
     ACT, score via per-batch M=1 matmuls into a [1, 1024] PSUM row
     (PE outputs must start at partition 0), folded to [8, 128] through
     a DRAM bounce (SBUF->SBUF reshape DMAs scramble partitions on HW),
     masked softmax on DVE/ACT.
  6. context via per-batch M=1 matmuls against the gathered enc windows.
  7. Indirect-scatter align_w into align_full and scatter-with-CCE-add
     into cum_new; ws_new via an iota/min argmax trick.
"""

import os
import sys
import types

import numpy as np

for _p in ("/root/.axon_site/_ro/trn_rl_repo", "/opt/trn_rl_repo"):
    if os.path.isdir(_p) and _p not in sys.path:
        sys.path.append(_p)

T, B, H, Q = 4096, 64, 128, 1024
K = 31
PAD = 15
W = 128
NCORES = 8
BC = B // NCORES          # batches per core
LP = 4224                 # loc_pad row length (15 + 4096 + zero tail)
CF = 256                  # chunk free-dim length; 8 rows * 16 chunks = 128
S8C = 144                 # packed [BC, S8C] i32 small-input tensor
S128C = 227               # packed [128, S128C] i32 small-input tensor

_CACHE = {}
last_results = None       # BassKernelResults of the most recent run


def _install_trace_hook():
    """Register the axon NTFF profiling hook so trace=True works."""
    try:
        import antenv.axon_hooks  # noqa: F401
        return
    except ImportError:
        pass
    mod = types.ModuleType("antenv.axon_hooks")
    _h = [None]
    mod.set_axon_ntff_profile_hook = lambda h: _h.__setitem__(0, h)
    mod.get_axon_ntff_profile_hook = lambda: _h[0]
    sys.modules["antenv.axon_hooks"] = mod
    try:
        from trn_agent_boot.trn_boot import _ntff_profile_via_ctypes

        hook = _ntff_profile_via_ctypes("/opt/axon/libaxon_pjrt.so")
        if hook is not None:
            mod.set_axon_ntff_profile_hook(hook)
    except Exception:
        pass


def _build_program():
    import concourse.bacc as bacc
    import concourse.bass as bass
    import concourse.mybir as mybir
    import concourse.tile as tile

    dt = mybir.dt
    f32, i32, u8, i16 = dt.float32, dt.int32, dt.uint8, dt.int16
    AX = mybir.AxisListType.X
    OP = mybir.AluOpType
    ACT = mybir.ActivationFunctionType

    nc = bacc.Bacc("TRN2", target_bir_lowering=False, debug=False,
                   num_devices=NCORES)

    enc = nc.dram_tensor("enc", [BC, T, H], f32, kind="ExternalInput")
    mask = nc.dram_tensor("mask", [BC, T], u8, kind="ExternalInput")
    cum = nc.dram_tensor("cum", [BC, T], f32, kind="ExternalInput")
    wq = nc.dram_tensor("wq", [H, Q], f32, kind="ExternalInput")
    sm8 = nc.dram_tensor("sm8", [BC, S8C], i32, kind="ExternalInput")
    sm128 = nc.dram_tensor("sm128", [128, S128C], i32, kind="ExternalInput")

    ctx_o = nc.dram_tensor("context", [BC, H], f32, kind="ExternalOutput")
    cumn_o = nc.dram_tensor("cum_new", [BC, T], f32, kind="ExternalOutput")
    alif_o = nc.dram_tensor("align_full", [BC, T], f32, kind="ExternalOutput")
    wsn_o = nc.dram_tensor("ws_new", [BC, 1], i32, kind="ExternalOutput")

    def chunked(ap):
        return ap.rearrange("b (c f) -> b c f", f=CF)

    with tile.TileContext(nc) as tc:
        with (
            tc.tile_pool(name="sb", bufs=1) as sb,
            tc.tile_pool(name="ps", bufs=1, space="PSUM") as ps,
            tc.tile_pool(name="dr", bufs=1, space="DRAM") as dr,
        ):
            # ---- warm the ACT function table off the critical path
            warm = sb.tile([1, 2], f32)
            nc.vector.memset(warm[:], 0.0)
            nc.scalar.activation(out=warm[:], in_=warm[:], func=ACT.Tanh)
            nc.scalar.activation(out=warm[:], in_=warm[:], func=ACT.Exp)

            # ---- two packed small-input loads; bitcast slice views
            s8 = sb.tile([BC, S8C], i32)
            nc.sync.dma_start(s8[:], sm8[:])
            s128 = sb.tile([128, S128C], i32)
            nc.sync.dma_start(s128[:], sm128[:])
            lidx_v = s8[:, 0:1]
            widx_v = s8[:, 1:2]
            ntok_v = s8[:, 2:3]
            wsf_v = s8[:, 3:4].bitcast(f32)
            initv_v = s8[:, 4:5].bitcast(f32)
            id8_v = s8[:, 8:16].bitcast(f32)
            iot_v = s8[:, 16:144].bitcast(f32)
            gidx_v = s128[:, 0:32].bitcast(i16)
            q_v = s128[:, 32:96].bitcast(f32)
            v_v = s128[:, 96:97].bitcast(f32)
            bhb_v = s128[:, 97:98].bitcast(f32)
            cwt_v = s128[0:31, 98:226].bitcast(f32)
            ones_v = s128[:, 226:227].bitcast(f32)



            # ---- bulk chunked cum/mask processing
            cum_sb = sb.tile([128, CF], f32)
            nc.sync.dma_start(cum_sb[:], chunked(cum[:]))
            mask_sb = sb.tile([128, CF], u8)
            nc.sync.dma_start(mask_sb[:], chunked(mask[:]))
            wq_sb = sb.tile([H, Q], f32)
            nc.sync.dma_start(wq_sb[:], wq[:])
            maskf = sb.tile([128, CF], f32)
            nc.vector.tensor_copy(maskf[:], mask_sb[:])
            cmask = sb.tile([128, CF], f32)
            nc.vector.tensor_mul(cmask[:], cum_sb[:], maskf[:])
            zc = sb.tile([128, CF], f32)
            nc.vector.memset(zc[:], 0.0)

            # bulk outputs (windows patched by indirect scatters below)
            nc.scalar.dma_start(chunked(alif_o[:]), zc[:])
            nc.scalar.dma_start(chunked(cumn_o[:]), cmask[:])

            # ---- loc_pad = [initial x PAD | masked cum | zeros]
            loc_pad = dr.tile([BC, LP], f32)
            nc.scalar.dma_start(chunked(loc_pad[:, PAD:PAD + T]), cmask[:])
            init15 = sb.tile([BC, PAD], f32)
            nc.vector.tensor_copy(init15[:], initv_v.to_broadcast([BC, PAD]))
            nc.scalar.dma_start(loc_pad[:, 0:PAD], init15[:])
            zt = sb.tile([BC, 128], f32)
            nc.vector.memset(zt[:], 0.0)
            nc.scalar.dma_start(loc_pad[:, PAD + T:LP], zt[:, :LP - PAD - T])

            # per-batch padded loc windows (one idx per partition, HW indirect
            # DMA streams the full dest row per index)
            locw = sb.tile([BC, 160], f32)
            locw_i = nc.gpsimd.indirect_dma_start(
                out=locw[:], out_offset=None, in_=loc_pad[:],
                in_offset=bass.IndirectOffsetOnAxis(ap=lidx_v, axis=1))
            locw_d = dr.tile([BC, 160], f32)
            nc.scalar.dma_start(locw_d[:], locw[:])
            # Toeplitz X[k, (b, w)] = locw[b, k + w] via overlapping DRAM AP
            X = sb.tile([K, BC * W], f32)
            x_src = bass.AP(locw_d[:].tensor, 0,
                            [[1, K], [160, BC], [1, W]])
            nc.sync.dma_start(X[:], x_src)
            # window mask gather (one idx per partition)
            mw_u8 = sb.tile([BC, W], u8)
            nc.gpsimd.indirect_dma_start(
                out=mw_u8[:], out_offset=None, in_=mask[:],
                in_offset=bass.IndirectOffsetOnAxis(ap=widx_v, axis=1))

            # ---- enc windows via dma_gather: idx i -> dst partition i%128
            # (=w), block i//128 (=b); row i reads enc[(b t), h] at t=ws_b+w.
            # The real gather costs ~9us of Q7 time (the cost model thinks
            # ~1us) and its result is only needed by the late context
            # matmuls, so force it AFTER the locw gather on the Pool queue.
            enc_sb = sb.tile([W, BC, H], f32)
            enc_g = nc.gpsimd.dma_gather(
                enc_sb[:], enc[:].rearrange("b t h -> (b t) h"), gidx_v,
                BC * W, BC * W, H)
            tile.add_dep_helper(enc_g.ins, locw_i.ins, sync=False,
                                reason="run the slow enc gather after locw")

            # ---- qp = Wq @ query (chunked over K=1024) + (bq + conv_b)
            qp_ps = ps.tile([H, BC], f32)
            for c in range(8):
                nc.tensor.matmul(
                    out=qp_ps[:], lhsT=wq_sb[:, c * 128:(c + 1) * 128],
                    rhs=q_v[:, c * BC:(c + 1) * BC],
                    start=(c == 0), stop=(c == 7))
            qpb = sb.tile([H, BC], f32)
            nc.vector.tensor_scalar(out=qpb[:], in0=qp_ps[:],
                                    scalar1=bhb_v, scalar2=None,
                                    op0=OP.add)

            # ---- conv as K=31 matmul over the Toeplitz gather
            feats_ps = ps.tile([H, BC * W], f32)
            nc.tensor.matmul(out=feats_ps[:, 0:512], lhsT=cwt_v,
                             rhs=X[:, 0:512], start=True, stop=True)
            nc.tensor.matmul(out=feats_ps[:, 512:1024], lhsT=cwt_v,
                             rhs=X[:, 512:1024], start=True, stop=True)

            # ---- tanh(feats + qp_b + bias)
            th = sb.tile([H, BC * W], f32)
            for b in range(BC):
                nc.scalar.activation(out=th[:, b * W:(b + 1) * W],
                                     in_=feats_ps[:, b * W:(b + 1) * W],
                                     func=ACT.Tanh, bias=qpb[:, b:b + 1],
                                     scale=1.0)

            # ---- score[b, w] = v . tanh_feats (per-batch M=1 matmuls into
            # a [1, BC*W] PSUM row; PE outputs must start at partition 0)
            score_ps = ps.tile([1, BC * W], f32)
            for b in range(BC):
                nc.tensor.matmul(out=score_ps[:, b * W:(b + 1) * W],
                                 lhsT=v_v,
                                 rhs=th[:, b * W:(b + 1) * W],
                                 start=True, stop=True)
            score_row = sb.tile([1, BC * W], f32)
            nc.vector.tensor_copy(score_row[:], score_ps[:])
            # SBUF->SBUF reshape DMAs scramble partitions on HW; bounce
            # through DRAM to fold [1, BC*W] into [BC, W].
            score_d = dr.tile([BC, W], f32)
            nc.scalar.dma_start(
                score_d[:].rearrange("b w -> (b w)")[None, :], score_row[:])
            score8 = sb.tile([BC, W], f32)
            nc.sync.dma_start(score8[:], score_d[:])

            # ---- mask + softmax over the window
            mwf = sb.tile([BC, W], f32)
            nc.vector.tensor_copy(mwf[:], mw_u8[:])
            pen = sb.tile([BC, W], f32)
            nc.vector.tensor_scalar(out=pen[:], in0=mwf[:], scalar1=1e30,
                                    scalar2=-1e30, op0=OP.mult, op1=OP.add)
            score_sb = sb.tile([BC, W], f32)
            nc.vector.tensor_add(score_sb[:], score8[:], pen[:])
            mx = sb.tile([BC, 1], f32)
            nc.vector.reduce_max(mx[:], score_sb[:], axis=AX)
            negmx = sb.tile([BC, 1], f32)
            nc.vector.tensor_scalar_mul(negmx[:], mx[:], -1.0)
            ex = sb.tile([BC, W], f32)
            nc.scalar.activation(out=ex[:], in_=score_sb[:], func=ACT.Exp,
                                 bias=negmx[:], scale=1.0)
            sm = sb.tile([BC, 1], f32)
            nc.vector.reduce_sum(sm[:], ex[:], axis=AX)
            rs = sb.tile([BC, 1], f32)
            nc.vector.reciprocal(rs[:], sm[:])
            aw = sb.tile([BC, W], f32)
            nc.vector.tensor_scalar(out=aw[:], in0=ex[:], scalar1=rs[:],
                                    scalar2=None, op0=OP.mult)

            # ---- transpose align_w -> [W, BC] for the context matmuls
            awT_ps = ps.tile([W, BC], f32)
            nc.tensor.transpose(out=awT_ps[:], in_=aw[:], identity=id8_v)
            awT = sb.tile([W, BC], f32)
            nc.vector.tensor_copy(awT[:], awT_ps[:])

            # ---- context[b, h] = sum_w aw[b, w] * enc[w, b, h]
            # ([1, BC*H] PSUM row == row-major [BC, H] when flattened)
            ctx_ps = ps.tile([1, BC * H], f32)
            for b in range(BC):
                nc.tensor.matmul(out=ctx_ps[:, b * H:(b + 1) * H],
                                 lhsT=awT[:, b:b + 1],
                                 rhs=enc_sb[:, b, :],
                                 start=True, stop=True)
            ctx_sb = sb.tile([1, BC * H], f32)
            nc.vector.tensor_copy(ctx_sb[:], ctx_ps[:])
            nc.sync.dma_start(
                ctx_o[:].rearrange("b h -> (b h)")[None, :], ctx_sb[:])

            # ---- scatter windows into the bulk outputs
            nc.gpsimd.indirect_dma_start(
                out=alif_o[:],
                out_offset=bass.IndirectOffsetOnAxis(ap=widx_v, axis=1),
                in_=aw[:], in_offset=None)
            nc.gpsimd.indirect_dma_start(
                out=cumn_o[:],
                out_offset=bass.IndirectOffsetOnAxis(ap=widx_v, axis=1),
                in_=aw[:], in_offset=None, compute_op=OP.add)

            # ---- ws_new = clip(ws + argmax(aw) - W//2, 0, ntok - W)
            amax = sb.tile([BC, 1], f32)
            nc.vector.reduce_max(amax[:], aw[:], axis=AX)
            eqm = sb.tile([BC, W], f32)
            nc.vector.tensor_scalar(out=eqm[:], in0=aw[:], scalar1=amax[:],
                                    scalar2=None, op0=OP.is_equal)
            t1 = sb.tile([BC, W], f32)
            nc.vector.tensor_scalar(out=t1[:], in0=eqm[:], scalar1=-4096.0,
                                    scalar2=4096.0, op0=OP.mult, op1=OP.add)
            t2 = sb.tile([BC, W], f32)
            nc.vector.tensor_mul(t2[:], iot_v, eqm[:])
            idxm = sb.tile([BC, W], f32)
            nc.vector.tensor_add(idxm[:], t1[:], t2[:])
            fidx = sb.tile([BC, 1], f32)
            nc.vector.tensor_reduce(fidx[:], idxm[:], axis=AX, op=OP.min)
            wsn = sb.tile([BC, 1], f32)
            nc.vector.tensor_scalar(out=wsn[:], in0=fidx[:],
                                    scalar1=wsf_v, scalar2=float(-(W // 2)),
                                    op0=OP.add, op1=OP.add)
            ntf = sb.tile([BC, 1], f32)
            nc.vector.tensor_copy(ntf[:], ntok_v)
            lim = sb.tile([BC, 1], f32)
            nc.vector.tensor_scalar_add(lim[:], ntf[:], float(-W))
            wsn2 = sb.tile([BC, 1], f32)
            nc.vector.tensor_tensor(wsn2[:], wsn[:], lim[:], op=OP.min)
            wsn3 = sb.tile([BC, 1], f32)
            nc.vector.tensor_scalar_max(wsn3[:], wsn2[:], 0.0)
            wsn_i = sb.tile([BC, 1], i32)
            nc.vector.tensor_copy(wsn_i[:], wsn3[:])
            nc.sync.dma_start(wsn_o[:], wsn_i[:])

    nc.compile()
    return nc


def _prep_core_inputs(core, encoded_tokens, tokens_mask, num_tokens, query,
                      cumulative_alignment, initial_cumulative_alignment,
                      window_start, wq_l, cwt_l, v_col, bhb_col):
    bs = slice(core * BC, (core + 1) * BC)
    ws = window_start[bs].astype(np.int64)

    qc = query[0, bs, :]                       # [BC, Q]
    q_l = np.ascontiguousarray(
        qc.T.reshape(8, 128, BC).transpose(1, 0, 2).reshape(128, 8 * BC))

    # dma_gather indices: unwrapped L[i] (i = b*W + w) = row b*T + ws_b + w
    # of enc viewed as [(b t), h]; wrapped layout [p, s] = L[s*16 + p%16],
    # replicated across the 8 GPSIMD cores (partition groups of 16).
    w_ar = np.arange(W, dtype=np.int64)
    L = (np.arange(BC, dtype=np.int64)[:, None] * T
         + ws[:, None] + w_ar[None, :]).reshape(-1)             # [BC*W]
    wrapped = L.reshape((BC * W) // 16, 16).T                   # [16, n/16]
    gidx = np.tile(wrapped, (8, 1)).astype(np.int16)            # [128, n/16]

    # packed [BC, S8C] i32 tensor
    s8 = np.zeros((BC, S8C), dtype=np.int32)
    s8[:, 0] = (np.arange(BC) * LP + ws).astype(np.int32)       # lidx
    s8[:, 1] = (np.arange(BC) * T + ws).astype(np.int32)        # widx
    s8[:, 2] = num_tokens[bs].astype(np.int32)                  # ntok
    s8[:, 3] = ws.astype(np.float32).view(np.int32)             # wsf
    s8[:, 4] = initial_cumulative_alignment[bs, 0].astype(
        np.float32).view(np.int32)                              # initv
    s8[:, 8:16] = np.eye(BC, dtype=np.float32).view(np.int32)   # id8
    s8[:, 16:144] = np.tile(np.arange(W, dtype=np.float32),
                            (BC, 1)).view(np.int32)             # iota

    # packed [128, S128C] i32 tensor
    s128 = np.zeros((128, S128C), dtype=np.int32)
    s128[:, 0:32] = gidx.view(np.int32)
    s128[:, 32:96] = q_l.astype(np.float32).view(np.int32)
    s128[:, 96] = v_col[:, 0].view(np.int32)
    s128[:, 97] = bhb_col[:, 0].view(np.int32)
    s128[0:31, 98:226] = cwt_l.view(np.int32)
    s128[:, 226] = np.float32(1.0).view(np.int32)

    return {
        "enc": np.ascontiguousarray(encoded_tokens[:, bs, :].transpose(1, 0, 2)),
        "mask": np.ascontiguousarray(tokens_mask[bs, :]).astype(np.uint8),
        "cum": np.ascontiguousarray(cumulative_alignment[bs, :]),
        "wq": wq_l,
        "sm8": s8,
        "sm128": s128,
    }


def make_in_maps(encoded_tokens, tokens_mask, num_tokens, query,
                 cumulative_alignment, initial_cumulative_alignment,
                 window_start, Wq, bq, conv_w, conv_b, v):
    encoded_tokens = np.asarray(encoded_tokens, dtype=np.float32)
    tokens_mask = np.asarray(tokens_mask)
    num_tokens = np.asarray(num_tokens, dtype=np.int32)
    query = np.asarray(query, dtype=np.float32)
    cumulative_alignment = np.asarray(cumulative_alignment, dtype=np.float32)
    initial_cumulative_alignment = np.asarray(
        initial_cumulative_alignment, dtype=np.float32)
    window_start = np.asarray(window_start, dtype=np.int32)
    Wq = np.asarray(Wq, dtype=np.float32)
    bq = np.asarray(bq, dtype=np.float32)
    conv_w = np.asarray(conv_w, dtype=np.float32)
    conv_b = np.asarray(conv_b, dtype=np.float32)
    v = np.asarray(v, dtype=np.float32)

    wq_l = np.ascontiguousarray(
        Wq.T.reshape(8, 128, H).transpose(1, 0, 2).reshape(128, Q))
    cwt_l = np.ascontiguousarray(conv_w[:, 0, :].T)     # [K, H]
    v_col = np.ascontiguousarray(v.reshape(H, 1))
    bhb_col = np.ascontiguousarray((bq + conv_b).reshape(H, 1))

    return [
        _prep_core_inputs(c, encoded_tokens, tokens_mask, num_tokens, query,
                          cumulative_alignment, initial_cumulative_alignment,
                          window_start, wq_l, cwt_l, v_col, bhb_col)
        for c in range(NCORES)
    ]


def kernel(**inputs):
    global last_results
    trace = bool(os.environ.get("KERNEL_TRACE"))
    if trace:
        _install_trace_hook()

    from concourse.bass_utils import run_bass_kernel_spmd

    if "nc" not in _CACHE:
        _CACHE["nc"] = _build_program()
    nc = _CACHE["nc"]

    in_maps = make_in_maps(**inputs)
    res = run_bass_kernel_spmd(nc, in_maps, core_ids=list(range(NCORES)),
                               trace=trace)
    last_results = res

    context = np.concatenate([r["context"] for r in res.results], axis=0)
    cum_new = np.concatenate([r["cum_new"] for r in res.results], axis=0)
    align_full = np.concatenate([r["align_full"] for r in res.results], axis=0)
    ws_new = np.concatenate(
        [r["ws_new"].reshape(BC) for r in res.results], axis=0)
    return context, cum_new, align_full, ws_new


# revision 47
# speedup vs baseline: 1.1047x; 1.0227x over previous
"""Location-sensitive attention Bass kernel for Trainium2 (8 NeuronCores).

Strategy: pure data parallel over the batch dim (64 batches -> 8 per core).
The computation is window-sparse: each batch row only touches a 128-wide
window of the 4096-long time axis, selected by `window_start`. All
data-dependent addressing is done with GPSIMD indirect DMAs / dma_gather
whose index tensors are computed on the host from `window_start` and fed as
inputs, so a single SPMD program serves all 8 cores.

Per-core device program:
  1. Two packed small-input loads (indices + parameters, bitcast views)
     so gathers can start early; enc windows via dma_gather (one 512B
     descriptor per (w, b), landing as [w, b, h] for the PE).
  2. cumulative_alignment + tokens_mask in a [128, 256] chunked layout
     (8 rows x 16 chunks -> 128 partitions), mask applied on DVE.
  3. Bulk-write align_full = zeros and cum_new = masked cum.
  4. Padded "loc" rows ([init x15 | masked cum | zeros]) in DRAM scratch,
     window-gathered to position 0, then the conv Toeplitz operand
     X[k, (b,w)] = loc[b, ws_b + k + w] loaded via an overlapping AP.
  5. qp = Wq @ query (PE, K=1024), conv as a K=31 matmul, tanh(+bias) on
     ACT, score via per-batch M=1 matmuls into a [1, 1024] PSUM row
     (PE outputs must start at partition 0), folded to [8, 128] through
     a DRAM bounce (SBUF->SBUF reshape DMAs scramble partitions on HW),
     masked softmax on DVE/ACT.
  6. context via per-batch M=1 matmuls against the gathered enc windows.
  7. Indirect-scatter align_w into align_full and scatter-with-CCE-add
     into cum_new; ws_new via an iota/min argmax trick.
"""

import os
import sys
import types

import numpy as np

for _p in ("/root/.axon_site/_ro/trn_rl_repo", "/opt/trn_rl_repo"):
    if os.path.isdir(_p) and _p not in sys.path:
        sys.path.append(_p)

T, B, H, Q = 4096, 64, 128, 1024
K = 31
PAD = 15
W = 128
NCORES = 8
BC = B // NCORES          # batches per core
LP = 4224                 # loc_pad row length (15 + 4096 + zero tail)
CF = 256                  # chunk free-dim length; 8 rows * 16 chunks = 128
S8C = 144                 # packed [BC, S8C] i32 small-input tensor
S128C = 227               # packed [128, S128C] i32 small-input tensor

_CACHE = {}
last_results = None       # BassKernelResults of the most recent run


def _install_trace_hook():
    """Register the axon NTFF profiling hook so trace=True works."""
    try:
        import antenv.axon_hooks  # noqa: F401
        return
    except ImportError:
        pass
    mod = types.ModuleType("antenv.axon_hooks")
    _h = [None]
    mod.set_axon_ntff_profile_hook = lambda h: _h.__setitem__(0, h)
    mod.get_axon_ntff_profile_hook = lambda: _h[0]
    sys.modules["antenv.axon_hooks"] = mod
    try:
        from trn_agent_boot.trn_boot import _ntff_profile_via_ctypes

        hook = _ntff_profile_via_ctypes("/opt/axon/libaxon_pjrt.so")
        if hook is not None:
            mod.set_axon_ntff_profile_hook(hook)
    except Exception:
        pass


def _build_program():
    import concourse.bacc as bacc
    import concourse.bass as bass
    import concourse.mybir as mybir
    import concourse.tile as tile

    dt = mybir.dt
    f32, i32, u8, i16 = dt.float32, dt.int32, dt.uint8, dt.int16
    AX = mybir.AxisListType.X
    OP = mybir.AluOpType
    ACT = mybir.ActivationFunctionType

    nc = bacc.Bacc("TRN2", target_bir_lowering=False, debug=False,
                   num_devices=NCORES)

    enc = nc.dram_tensor("enc", [BC, T, H], f32, kind="ExternalInput")
    mask = nc.dram_tensor("mask", [BC, T], u8, kind="ExternalInput")
    cum = nc.dram_tensor("cum", [BC, T], f32, kind="ExternalInput")
    wq = nc.dram_tensor("wq", [H, Q], f32, kind="ExternalInput")
    sm8 = nc.dram_tensor("sm8", [BC, S8C], i32, kind="ExternalInput")
    sm128 = nc.dram_tensor("sm128", [128, S128C], i32, kind="ExternalInput")

    ctx_o = nc.dram_tensor("context", [BC, H], f32, kind="ExternalOutput")
    cumn_o = nc.dram_tensor("cum_new", [BC, T], f32, kind="ExternalOutput")
    alif_o = nc.dram_tensor("align_full", [BC, T], f32, kind="ExternalOutput")
    wsn_o = nc.dram_tensor("ws_new", [BC, 1], i32, kind="ExternalOutput")

    def chunked(ap):
        return ap.rearrange("b (c f) -> b c f", f=CF)

    with tile.TileContext(nc) as tc:
        with (
            tc.tile_pool(name="sb", bufs=1) as sb,
            tc.tile_pool(name="ps", bufs=1, space="PSUM") as ps,
            tc.tile_pool(name="dr", bufs=1, space="DRAM") as dr,
        ):
            # ---- warm the ACT function table off the critical path
            warm = sb.tile([1, 2], f32)
            nc.vector.memset(warm[:], 0.0)
            nc.scalar.activation(out=warm[:], in_=warm[:], func=ACT.Tanh)
            nc.scalar.activation(out=warm[:], in_=warm[:], func=ACT.Exp)

            # ---- two packed small-input loads; bitcast slice views
            s8 = sb.tile([BC, S8C], i32)
            nc.sync.dma_start(s8[:], sm8[:])
            s128 = sb.tile([128, S128C], i32)
            nc.sync.dma_start(s128[:], sm128[:])
            lidx_v = s8[:, 0:1]
            widx_v = s8[:, 1:2]
            ntok_v = s8[:, 2:3]
            wsf_v = s8[:, 3:4].bitcast(f32)
            initv_v = s8[:, 4:5].bitcast(f32)
            id8_v = s8[:, 8:16].bitcast(f32)
            iot_v = s8[:, 16:144].bitcast(f32)
            gidx_v = s128[:, 0:32].bitcast(i16)
            q_v = s128[:, 32:96].bitcast(f32)
            v_v = s128[:, 96:97].bitcast(f32)
            bhb_v = s128[:, 97:98].bitcast(f32)
            cwt_v = s128[0:31, 98:226].bitcast(f32)
            ones_v = s128[:, 226:227].bitcast(f32)



            # ---- bulk chunked cum/mask processing
            cum_sb = sb.tile([128, CF], f32)
            nc.sync.dma_start(cum_sb[:], chunked(cum[:]))
            mask_sb = sb.tile([128, CF], u8)
            nc.sync.dma_start(mask_sb[:], chunked(mask[:]))
            wq_sb = sb.tile([H, Q], f32)
            nc.sync.dma_start(wq_sb[:], wq[:])
            maskf = sb.tile([128, CF], f32)
            nc.vector.tensor_copy(maskf[:], mask_sb[:])
            cmask = sb.tile([128, CF], f32)
            nc.vector.tensor_mul(cmask[:], cum_sb[:], maskf[:])
            zc = sb.tile([128, CF], f32)
            nc.vector.memset(zc[:], 0.0)

            # bulk outputs (windows patched by indirect scatters below)
            nc.scalar.dma_start(chunked(alif_o[:]), zc[:])
            nc.scalar.dma_start(chunked(cumn_o[:]), cmask[:])

            # ---- loc_pad = [initial x PAD | masked cum | zeros]
            loc_pad = dr.tile([BC, LP], f32)
            nc.scalar.dma_start(chunked(loc_pad[:, PAD:PAD + T]), cmask[:])
            init15 = sb.tile([BC, PAD], f32)
            nc.vector.tensor_copy(init15[:], initv_v.to_broadcast([BC, PAD]))
            nc.scalar.dma_start(loc_pad[:, 0:PAD], init15[:])
            zt = sb.tile([BC, 128], f32)
            nc.vector.memset(zt[:], 0.0)
            nc.scalar.dma_start(loc_pad[:, PAD + T:LP], zt[:, :LP - PAD - T])

            # per-batch padded loc windows (one idx per partition, HW indirect
            # DMA streams the full dest row per index)
            locw = sb.tile([BC, 160], f32)
            locw_i = nc.gpsimd.indirect_dma_start(
                out=locw[:], out_offset=None, in_=loc_pad[:],
                in_offset=bass.IndirectOffsetOnAxis(ap=lidx_v, axis=1))
            locw_d = dr.tile([BC, 160], f32)
            nc.scalar.dma_start(locw_d[:], locw[:])
            # Toeplitz X[k, (b, w)] = locw[b, k + w] via overlapping DRAM AP
            X = sb.tile([K, BC * W], f32)
            x_src = bass.AP(locw_d[:].tensor, 0,
                            [[1, K], [160, BC], [1, W]])
            nc.sync.dma_start(X[:], x_src)
            # window mask gather (one idx per partition)
            mw_u8 = sb.tile([BC, W], u8)
            nc.gpsimd.indirect_dma_start(
                out=mw_u8[:], out_offset=None, in_=mask[:],
                in_offset=bass.IndirectOffsetOnAxis(ap=widx_v, axis=1))

            # ---- enc windows via dma_gather: idx i -> dst partition i%128
            # (=w), block i//128 (=b); row i reads enc[(b t), h] at t=ws_b+w.
            # The real gather costs ~9us of Q7 time (the cost model thinks
            # ~1us) and its result is only needed by the late context
            # matmuls, so force it AFTER the locw gather on the Pool queue.
            enc_sb = sb.tile([W, BC, H], f32)
            enc_g = nc.gpsimd.dma_gather(
                enc_sb[:], enc[:].rearrange("b t h -> (b t) h"), gidx_v,
                BC * W, BC * W, H)
            tile.add_dep_helper(enc_g.ins, locw_i.ins, sync=False,
                                reason="run the slow enc gather after locw")

            # ---- qp = Wq @ query (chunked over K=1024) + (bq + conv_b)
            qp_ps = ps.tile([H, BC], f32)
            for c in range(8):
                nc.tensor.matmul(
                    out=qp_ps[:], lhsT=wq_sb[:, c * 128:(c + 1) * 128],
                    rhs=q_v[:, c * BC:(c + 1) * BC],
                    start=(c == 0), stop=(c == 7))
            qpb = sb.tile([H, BC], f32)
            nc.vector.tensor_scalar(out=qpb[:], in0=qp_ps[:],
                                    scalar1=bhb_v, scalar2=None,
                                    op0=OP.add)

            # ---- conv as K=31 matmul over the Toeplitz gather
            feats_ps = ps.tile([H, BC * W], f32)
            nc.tensor.matmul(out=feats_ps[:, 0:512], lhsT=cwt_v,
                             rhs=X[:, 0:512], start=True, stop=True)
            nc.tensor.matmul(out=feats_ps[:, 512:1024], lhsT=cwt_v,
                             rhs=X[:, 512:1024], start=True, stop=True)

            # ---- tanh(feats + qp_b + bias)
            th = sb.tile([H, BC * W], f32)
            for b in range(BC):
                nc.scalar.activation(out=th[:, b * W:(b + 1) * W],
                                     in_=feats_ps[:, b * W:(b + 1) * W],
                                     func=ACT.Tanh, bias=qpb[:, b:b + 1],
                                     scale=1.0)

            # ---- score[b, w] = v . tanh_feats (per-batch M=1 matmuls into
            # a [1, BC*W] PSUM row; PE outputs must start at partition 0)
            score_ps = ps.tile([1, BC * W], f32)
            for b in range(BC):
                nc.tensor.matmul(out=score_ps[:, b * W:(b + 1) * W],
                                 lhsT=v_v,
                                 rhs=th[:, b * W:(b + 1) * W],
                                 start=True, stop=True)
            score_row = sb.tile([1, BC * W], f32)
            nc.vector.tensor_copy(score_row[:, 0:512], score_ps[:, 0:512])
            nc.scalar.copy(out=score_row[:, 512:1024],
                           in_=score_ps[:, 512:1024])
            # SBUF->SBUF reshape DMAs scramble partitions on HW; bounce
            # through DRAM to fold [1, BC*W] into [BC, W].
            score_d = dr.tile([BC, W], f32)
            nc.scalar.dma_start(
                score_d[:].rearrange("b w -> (b w)")[None, :], score_row[:])
            score8 = sb.tile([BC, W], f32)
            nc.sync.dma_start(score8[:], score_d[:])

            # ---- mask + softmax over the window
            mwf = sb.tile([BC, W], f32)
            nc.vector.tensor_copy(mwf[:], mw_u8[:])
            pen = sb.tile([BC, W], f32)
            nc.vector.tensor_scalar(out=pen[:], in0=mwf[:], scalar1=1e30,
                                    scalar2=-1e30, op0=OP.mult, op1=OP.add)
            score_sb = sb.tile([BC, W], f32)
            nc.vector.tensor_add(score_sb[:], score8[:], pen[:])
            mx = sb.tile([BC, 1], f32)
            nc.vector.reduce_max(mx[:], score_sb[:], axis=AX)
            negmx = sb.tile([BC, 1], f32)
            nc.vector.tensor_scalar_mul(negmx[:], mx[:], -1.0)
            ex = sb.tile([BC, W], f32)
            nc.scalar.activation(out=ex[:], in_=score_sb[:], func=ACT.Exp,
                                 bias=negmx[:], scale=1.0)
            sm = sb.tile([BC, 1], f32)
            nc.vector.reduce_sum(sm[:], ex[:], axis=AX)
            rs = sb.tile([BC, 1], f32)
            nc.vector.reciprocal(rs[:], sm[:])
            aw = sb.tile([BC, W], f32)
            nc.vector.tensor_scalar(out=aw[:], in0=ex[:], scalar1=rs[:],
                                    scalar2=None, op0=OP.mult)

            # ---- transpose align_w -> [W, BC] for the context matmuls
            awT_ps = ps.tile([W, BC], f32)
            nc.tensor.transpose(out=awT_ps[:], in_=aw[:], identity=id8_v)
            awT = sb.tile([W, BC], f32)
            nc.vector.tensor_copy(awT[:], awT_ps[:])

            # ---- context[b, h] = sum_w aw[b, w] * enc[w, b, h]
            # ([1, BC*H] PSUM row == row-major [BC, H] when flattened)
            ctx_ps = ps.tile([1, BC * H], f32)
            for b in range(BC):
                nc.tensor.matmul(out=ctx_ps[:, b * H:(b + 1) * H],
                                 lhsT=awT[:, b:b + 1],
                                 rhs=enc_sb[:, b, :],
                                 start=True, stop=True)
            ctx_sb = sb.tile([1, BC * H], f32)
            nc.vector.tensor_copy(ctx_sb[:, 0:512], ctx_ps[:, 0:512])
            nc.scalar.copy(out=ctx_sb[:, 512:1024], in_=ctx_ps[:, 512:1024])
            nc.sync.dma_start(
                ctx_o[:].rearrange("b h -> (b h)")[None, :], ctx_sb[:])

            # ---- scatter windows into the bulk outputs
            nc.gpsimd.indirect_dma_start(
                out=alif_o[:],
                out_offset=bass.IndirectOffsetOnAxis(ap=widx_v, axis=1),
                in_=aw[:], in_offset=None)
            nc.gpsimd.indirect_dma_start(
                out=cumn_o[:],
                out_offset=bass.IndirectOffsetOnAxis(ap=widx_v, axis=1),
                in_=aw[:], in_offset=None, compute_op=OP.add)

            # ---- ws_new = clip(ws + argmax(aw) - W//2, 0, ntok - W)
            amax = sb.tile([BC, 1], f32)
            nc.vector.reduce_max(amax[:], aw[:], axis=AX)
            eqm = sb.tile([BC, W], f32)
            nc.vector.tensor_scalar(out=eqm[:], in0=aw[:], scalar1=amax[:],
                                    scalar2=None, op0=OP.is_equal)
            t1 = sb.tile([BC, W], f32)
            nc.vector.tensor_scalar(out=t1[:], in0=eqm[:], scalar1=-4096.0,
                                    scalar2=4096.0, op0=OP.mult, op1=OP.add)
            t2 = sb.tile([BC, W], f32)
            nc.vector.tensor_mul(t2[:], iot_v, eqm[:])
            idxm = sb.tile([BC, W], f32)
            nc.vector.tensor_add(idxm[:], t1[:], t2[:])
            fidx = sb.tile([BC, 1], f32)
            nc.vector.tensor_reduce(fidx[:], idxm[:], axis=AX, op=OP.min)
            wsn = sb.tile([BC, 1], f32)
            nc.vector.tensor_scalar(out=wsn[:], in0=fidx[:],
                                    scalar1=wsf_v, scalar2=float(-(W // 2)),
                                    op0=OP.add, op1=OP.add)
            ntf = sb.tile([BC, 1], f32)
            nc.vector.tensor_copy(ntf[:], ntok_v)
            lim = sb.tile([BC, 1], f32)
            nc.vector.tensor_scalar_add(lim[:], ntf[:], float(-W))
            wsn2 = sb.tile([BC, 1], f32)
            nc.vector.tensor_tensor(wsn2[:], wsn[:], lim[:], op=OP.min)
            wsn3 = sb.tile([BC, 1], f32)
            nc.vector.tensor_scalar_max(wsn3[:], wsn2[:], 0.0)
            wsn_i = sb.tile([BC, 1], i32)
            nc.vector.tensor_copy(wsn_i[:], wsn3[:])
            nc.sync.dma_start(wsn_o[:], wsn_i[:])

    nc.compile()
    return nc


def _prep_core_inputs(core, encoded_tokens, tokens_mask, num_tokens, query,
                      cumulative_alignment, initial_cumulative_alignment,
                      window_start, wq_l, cwt_l, v_col, bhb_col):
    bs = slice(core * BC, (core + 1) * BC)
    ws = window_start[bs].astype(np.int64)

    qc = query[0, bs, :]                       # [BC, Q]
    q_l = np.ascontiguousarray(
        qc.T.reshape(8, 128, BC).transpose(1, 0, 2).reshape(128, 8 * BC))

    # dma_gather indices: unwrapped L[i] (i = b*W + w) = row b*T + ws_b + w
    # of enc viewed as [(b t), h]; wrapped layout [p, s] = L[s*16 + p%16],
    # replicated across the 8 GPSIMD cores (partition groups of 16).
    w_ar = np.arange(W, dtype=np.int64)
    L = (np.arange(BC, dtype=np.int64)[:, None] * T
         + ws[:, None] + w_ar[None, :]).reshape(-1)             # [BC*W]
    wrapped = L.reshape((BC * W) // 16, 16).T                   # [16, n/16]
    gidx = np.tile(wrapped, (8, 1)).astype(np.int16)            # [128, n/16]

    # packed [BC, S8C] i32 tensor
    s8 = np.zeros((BC, S8C), dtype=np.int32)
    s8[:, 0] = (np.arange(BC) * LP + ws).astype(np.int32)       # lidx
    s8[:, 1] = (np.arange(BC) * T + ws).astype(np.int32)        # widx
    s8[:, 2] = num_tokens[bs].astype(np.int32)                  # ntok
    s8[:, 3] = ws.astype(np.float32).view(np.int32)             # wsf
    s8[:, 4] = initial_cumulative_alignment[bs, 0].astype(
        np.float32).view(np.int32)                              # initv
    s8[:, 8:16] = np.eye(BC, dtype=np.float32).view(np.int32)   # id8
    s8[:, 16:144] = np.tile(np.arange(W, dtype=np.float32),
                            (BC, 1)).view(np.int32)             # iota

    # packed [128, S128C] i32 tensor
    s128 = np.zeros((128, S128C), dtype=np.int32)
    s128[:, 0:32] = gidx.view(np.int32)
    s128[:, 32:96] = q_l.astype(np.float32).view(np.int32)
    s128[:, 96] = v_col[:, 0].view(np.int32)
    s128[:, 97] = bhb_col[:, 0].view(np.int32)
    s128[0:31, 98:226] = cwt_l.view(np.int32)
    s128[:, 226] = np.float32(1.0).view(np.int32)

    return {
        "enc": np.ascontiguousarray(encoded_tokens[:, bs, :].transpose(1, 0, 2)),
        "mask": np.ascontiguousarray(tokens_mask[bs, :]).astype(np.uint8),
        "cum": np.ascontiguousarray(cumulative_alignment[bs, :]),
        "wq": wq_l,
        "sm8": s8,
        "sm128": s128,
    }


def make_in_maps(encoded_tokens, tokens_mask, num_tokens, query,
                 cumulative_alignment, initial_cumulative_alignment,
                 window_start, Wq, bq, conv_w, conv_b, v):
    encoded_tokens = np.asarray(encoded_tokens, dtype=np.float32)
    tokens_mask = np.asarray(tokens_mask)
    num_tokens = np.asarray(num_tokens, dtype=np.int32)
    query = np.asarray(query, dtype=np.float32)
    cumulative_alignment = np.asarray(cumulative_alignment, dtype=np.float32)
    initial_cumulative_alignment = np.asarray(
        initial_cumulative_alignment, dtype=np.float32)
    window_start = np.asarray(window_start, dtype=np.int32)
    Wq = np.asarray(Wq, dtype=np.float32)
    bq = np.asarray(bq, dtype=np.float32)
    conv_w = np.asarray(conv_w, dtype=np.float32)
    conv_b = np.asarray(conv_b, dtype=np.float32)
    v = np.asarray(v, dtype=np.float32)

    wq_l = np.ascontiguousarray(
        Wq.T.reshape(8, 128, H).transpose(1, 0, 2).reshape(128, Q))
    cwt_l = np.ascontiguousarray(conv_w[:, 0, :].T)     # [K, H]
    v_col = np.ascontiguousarray(v.reshape(H, 1))
    bhb_col = np.ascontiguousarray((bq + conv_b).reshape(H, 1))

    return [
        _prep_core_inputs(c, encoded_tokens, tokens_mask, num_tokens, query,
                          cumulative_alignment, initial_cumulative_alignment,
                          window_start, wq_l, cwt_l, v_col, bhb_col)
        for c in range(NCORES)
    ]


def kernel(**inputs):
    global last_results
    trace = bool(os.environ.get("KERNEL_TRACE"))
    if trace:
        _install_trace_hook()

    from concourse.bass_utils import run_bass_kernel_spmd

    if "nc" not in _CACHE:
        _CACHE["nc"] = _build_program()
    nc = _CACHE["nc"]

    in_maps = make_in_maps(**inputs)
    res = run_bass_kernel_spmd(nc, in_maps, core_ids=list(range(NCORES)),
                               trace=trace)
    last_results = res

    context = np.concatenate([r["context"] for r in res.results], axis=0)
    cum_new = np.concatenate([r["cum_new"] for r in res.results], axis=0)
    align_full = np.concatenate([r["align_full"] for r in res.results], axis=0)
    ws_new = np.concatenate(
        [r["ws_new"].reshape(BC) for r in res.results], axis=0)
    return context, cum_new, align_full, ws_new


# revision 48
# speedup vs baseline: 1.1173x; 1.0114x over previous
"""Location-sensitive attention Bass kernel for Trainium2 (8 NeuronCores).

Strategy: pure data parallel over the batch dim (64 batches -> 8 per core).
The computation is window-sparse: each batch row only touches a 128-wide
window of the 4096-long time axis, selected by `window_start`. All
data-dependent addressing is done with GPSIMD indirect DMAs / dma_gather
whose index tensors are computed on the host from `window_start` and fed as
inputs, so a single SPMD program serves all 8 cores.

Per-core device program:
  1. Two packed small-input loads (indices + parameters, bitcast views)
     so gathers can start early; enc windows via dma_gather (one 512B
     descriptor per (w, b), landing as [w, b, h] for the PE).
  2. cumulative_alignment + tokens_mask in a [128, 256] chunked layout
     (8 rows x 16 chunks -> 128 partitions), mask applied on DVE.
  3. Bulk-write align_full = zeros and cum_new = masked cum.
  4. Padded "loc" rows ([init x15 | masked cum | zeros]) in DRAM scratch,
     window-gathered to position 0, then the conv Toeplitz operand
     X[k, (b,w)] = loc[b, ws_b + k + w] loaded via an overlapping AP.
  5. qp = Wq @ query (PE, K=1024), conv as a K=31 matmul, tanh(+bias) on
     ACT, score via per-batch M=1 matmuls into a [1, 1024] PSUM row
     (PE outputs must start at partition 0), folded to [8, 128] through
     a DRAM bounce (SBUF->SBUF reshape DMAs scramble partitions on HW),
     masked softmax on DVE/ACT.
  6. context via per-batch M=1 matmuls against the gathered enc windows.
  7. Indirect-scatter align_w into align_full and scatter-with-CCE-add
     into cum_new; ws_new via an iota/min argmax trick.
"""

import os
import sys
import types

import numpy as np

for _p in ("/root/.axon_site/_ro/trn_rl_repo", "/opt/trn_rl_repo"):
    if os.path.isdir(_p) and _p not in sys.path:
        sys.path.append(_p)

T, B, H, Q = 4096, 64, 128, 1024
K = 31
PAD = 15
W = 128
NCORES = 8
BC = B // NCORES          # batches per core
LP = 4224                 # loc_pad row length (15 + 4096 + zero tail)
CF = 256                  # chunk free-dim length; 8 rows * 16 chunks = 128
S8C = 144                 # packed [BC, S8C] i32 small-input tensor
S128C = 227               # packed [128, S128C] i32 small-input tensor

_CACHE = {}
last_results = None       # BassKernelResults of the most recent run


def _install_trace_hook():
    """Register the axon NTFF profiling hook so trace=True works."""
    try:
        import antenv.axon_hooks  # noqa: F401
        return
    except ImportError:
        pass
    mod = types.ModuleType("antenv.axon_hooks")
    _h = [None]
    mod.set_axon_ntff_profile_hook = lambda h: _h.__setitem__(0, h)
    mod.get_axon_ntff_profile_hook = lambda: _h[0]
    sys.modules["antenv.axon_hooks"] = mod
    try:
        from trn_agent_boot.trn_boot import _ntff_profile_via_ctypes

        hook = _ntff_profile_via_ctypes("/opt/axon/libaxon_pjrt.so")
        if hook is not None:
            mod.set_axon_ntff_profile_hook(hook)
    except Exception:
        pass


def _build_program():
    import concourse.bacc as bacc
    import concourse.bass as bass
    import concourse.mybir as mybir
    import concourse.tile as tile

    dt = mybir.dt
    f32, i32, u8, i16 = dt.float32, dt.int32, dt.uint8, dt.int16
    AX = mybir.AxisListType.X
    OP = mybir.AluOpType
    ACT = mybir.ActivationFunctionType

    nc = bacc.Bacc("TRN2", target_bir_lowering=False, debug=False,
                   num_devices=NCORES)

    enc = nc.dram_tensor("enc", [BC, T, H], f32, kind="ExternalInput")
    mask = nc.dram_tensor("mask", [BC, T], u8, kind="ExternalInput")
    cum = nc.dram_tensor("cum", [BC, T], f32, kind="ExternalInput")
    wq = nc.dram_tensor("wq", [H, Q], f32, kind="ExternalInput")
    sm8 = nc.dram_tensor("sm8", [BC, S8C], i32, kind="ExternalInput")
    sm128 = nc.dram_tensor("sm128", [128, S128C], i32, kind="ExternalInput")

    ctx_o = nc.dram_tensor("context", [BC, H], f32, kind="ExternalOutput")
    cumn_o = nc.dram_tensor("cum_new", [BC, T], f32, kind="ExternalOutput")
    alif_o = nc.dram_tensor("align_full", [BC, T], f32, kind="ExternalOutput")
    wsn_o = nc.dram_tensor("ws_new", [BC, 1], i32, kind="ExternalOutput")

    def chunked(ap):
        return ap.rearrange("b (c f) -> b c f", f=CF)

    with tile.TileContext(nc) as tc:
        with (
            tc.tile_pool(name="sb", bufs=1) as sb,
            tc.tile_pool(name="ps", bufs=1, space="PSUM") as ps,
            tc.tile_pool(name="dr", bufs=1, space="DRAM") as dr,
        ):
            # ---- warm the ACT function table off the critical path
            warm = sb.tile([1, 2], f32)
            nc.vector.memset(warm[:], 0.0)
            nc.scalar.activation(out=warm[:], in_=warm[:], func=ACT.Tanh)
            nc.scalar.activation(out=warm[:], in_=warm[:], func=ACT.Exp)

            # ---- critical-chain loads first: cum/mask feed the conv path
            cum_sb = sb.tile([128, CF], f32)
            nc.sync.dma_start(cum_sb[:], chunked(cum[:]))
            mask_sb = sb.tile([128, CF], u8)
            nc.sync.dma_start(mask_sb[:], chunked(mask[:]))
            # ---- two packed small-input loads; bitcast slice views
            s8 = sb.tile([BC, S8C], i32)
            nc.sync.dma_start(s8[:], sm8[:])
            s128 = sb.tile([128, S128C], i32)
            nc.sync.dma_start(s128[:], sm128[:])
            lidx_v = s8[:, 0:1]
            widx_v = s8[:, 1:2]
            ntok_v = s8[:, 2:3]
            wsf_v = s8[:, 3:4].bitcast(f32)
            initv_v = s8[:, 4:5].bitcast(f32)
            id8_v = s8[:, 8:16].bitcast(f32)
            iot_v = s8[:, 16:144].bitcast(f32)
            gidx_v = s128[:, 0:32].bitcast(i16)
            q_v = s128[:, 32:96].bitcast(f32)
            v_v = s128[:, 96:97].bitcast(f32)
            bhb_v = s128[:, 97:98].bitcast(f32)
            cwt_v = s128[0:31, 98:226].bitcast(f32)
            ones_v = s128[:, 226:227].bitcast(f32)



            # ---- bulk chunked cum/mask processing
            wq_sb = sb.tile([H, Q], f32)
            nc.sync.dma_start(wq_sb[:], wq[:])
            maskf = sb.tile([128, CF], f32)
            nc.vector.tensor_copy(maskf[:], mask_sb[:])
            cmask = sb.tile([128, CF], f32)
            nc.vector.tensor_mul(cmask[:], cum_sb[:], maskf[:])
            zc = sb.tile([128, CF], f32)
            nc.vector.memset(zc[:], 0.0)

            # bulk outputs (windows patched by indirect scatters below)
            nc.scalar.dma_start(chunked(alif_o[:]), zc[:])
            nc.scalar.dma_start(chunked(cumn_o[:]), cmask[:])

            # ---- loc_pad = [initial x PAD | masked cum | zeros]
            loc_pad = dr.tile([BC, LP], f32)
            nc.scalar.dma_start(chunked(loc_pad[:, PAD:PAD + T]), cmask[:])
            init15 = sb.tile([BC, PAD], f32)
            nc.vector.tensor_copy(init15[:], initv_v.to_broadcast([BC, PAD]))
            nc.scalar.dma_start(loc_pad[:, 0:PAD], init15[:])
            zt = sb.tile([BC, 128], f32)
            nc.vector.memset(zt[:], 0.0)
            nc.scalar.dma_start(loc_pad[:, PAD + T:LP], zt[:, :LP - PAD - T])

            # per-batch padded loc windows (one idx per partition, HW indirect
            # DMA streams the full dest row per index)
            locw = sb.tile([BC, 160], f32)
            locw_i = nc.gpsimd.indirect_dma_start(
                out=locw[:], out_offset=None, in_=loc_pad[:],
                in_offset=bass.IndirectOffsetOnAxis(ap=lidx_v, axis=1))
            locw_d = dr.tile([BC, 160], f32)
            nc.scalar.dma_start(locw_d[:], locw[:])
            # Toeplitz X[k, (b, w)] = locw[b, k + w] via overlapping DRAM AP
            X = sb.tile([K, BC * W], f32)
            x_src = bass.AP(locw_d[:].tensor, 0,
                            [[1, K], [160, BC], [1, W]])
            nc.sync.dma_start(X[:], x_src)
            # window mask gather (one idx per partition)
            mw_u8 = sb.tile([BC, W], u8)
            nc.gpsimd.indirect_dma_start(
                out=mw_u8[:], out_offset=None, in_=mask[:],
                in_offset=bass.IndirectOffsetOnAxis(ap=widx_v, axis=1))

            # ---- enc windows via dma_gather: idx i -> dst partition i%128
            # (=w), block i//128 (=b); row i reads enc[(b t), h] at t=ws_b+w.
            # The real gather costs ~9us of Q7 time (the cost model thinks
            # ~1us) and its result is only needed by the late context
            # matmuls, so force it AFTER the locw gather on the Pool queue.
            enc_sb = sb.tile([W, BC, H], f32)
            enc_g = nc.gpsimd.dma_gather(
                enc_sb[:], enc[:].rearrange("b t h -> (b t) h"), gidx_v,
                BC * W, BC * W, H)
            tile.add_dep_helper(enc_g.ins, locw_i.ins, sync=False,
                                reason="run the slow enc gather after locw")

            # ---- qp = Wq @ query (chunked over K=1024) + (bq + conv_b)
            qp_ps = ps.tile([H, BC], f32)
            for c in range(8):
                nc.tensor.matmul(
                    out=qp_ps[:], lhsT=wq_sb[:, c * 128:(c + 1) * 128],
                    rhs=q_v[:, c * BC:(c + 1) * BC],
                    start=(c == 0), stop=(c == 7))
            qpb = sb.tile([H, BC], f32)
            nc.vector.tensor_scalar(out=qpb[:], in0=qp_ps[:],
                                    scalar1=bhb_v, scalar2=None,
                                    op0=OP.add)

            # ---- conv as K=31 matmul over the Toeplitz gather
            feats_ps = ps.tile([H, BC * W], f32)
            nc.tensor.matmul(out=feats_ps[:, 0:512], lhsT=cwt_v,
                             rhs=X[:, 0:512], start=True, stop=True)
            nc.tensor.matmul(out=feats_ps[:, 512:1024], lhsT=cwt_v,
                             rhs=X[:, 512:1024], start=True, stop=True)

            # ---- tanh(feats + qp_b + bias)
            th = sb.tile([H, BC * W], f32)
            for b in range(BC):
                nc.scalar.activation(out=th[:, b * W:(b + 1) * W],
                                     in_=feats_ps[:, b * W:(b + 1) * W],
                                     func=ACT.Tanh, bias=qpb[:, b:b + 1],
                                     scale=1.0)

            # ---- score[b, w] = v . tanh_feats (per-batch M=1 matmuls into
            # a [1, BC*W] PSUM row; PE outputs must start at partition 0)
            score_ps = ps.tile([1, BC * W], f32)
            for b in range(BC):
                nc.tensor.matmul(out=score_ps[:, b * W:(b + 1) * W],
                                 lhsT=v_v,
                                 rhs=th[:, b * W:(b + 1) * W],
                                 start=True, stop=True)
            score_row = sb.tile([1, BC * W], f32)
            nc.vector.tensor_copy(score_row[:, 0:512], score_ps[:, 0:512])
            nc.scalar.copy(out=score_row[:, 512:1024],
                           in_=score_ps[:, 512:1024])
            # SBUF->SBUF reshape DMAs scramble partitions on HW; bounce
            # through DRAM to fold [1, BC*W] into [BC, W].
            score_d = dr.tile([BC, W], f32)
            nc.scalar.dma_start(
                score_d[:].rearrange("b w -> (b w)")[None, :], score_row[:])
            score8 = sb.tile([BC, W], f32)
            nc.sync.dma_start(score8[:], score_d[:])

            # ---- mask + softmax over the window
            mwf = sb.tile([BC, W], f32)
            nc.vector.tensor_copy(mwf[:], mw_u8[:])
            pen = sb.tile([BC, W], f32)
            nc.vector.tensor_scalar(out=pen[:], in0=mwf[:], scalar1=1e30,
                                    scalar2=-1e30, op0=OP.mult, op1=OP.add)
            score_sb = sb.tile([BC, W], f32)
            nc.vector.tensor_add(score_sb[:], score8[:], pen[:])
            mx = sb.tile([BC, 1], f32)
            nc.vector.reduce_max(mx[:], score_sb[:], axis=AX)
            negmx = sb.tile([BC, 1], f32)
            nc.vector.tensor_scalar_mul(negmx[:], mx[:], -1.0)
            ex = sb.tile([BC, W], f32)
            nc.scalar.activation(out=ex[:], in_=score_sb[:], func=ACT.Exp,
                                 bias=negmx[:], scale=1.0)
            sm = sb.tile([BC, 1], f32)
            nc.vector.reduce_sum(sm[:], ex[:], axis=AX)
            rs = sb.tile([BC, 1], f32)
            nc.vector.reciprocal(rs[:], sm[:])
            aw = sb.tile([BC, W], f32)
            nc.vector.tensor_scalar(out=aw[:], in0=ex[:], scalar1=rs[:],
                                    scalar2=None, op0=OP.mult)

            # ---- transpose align_w -> [W, BC] for the context matmuls
            awT_ps = ps.tile([W, BC], f32)
            nc.tensor.transpose(out=awT_ps[:], in_=aw[:], identity=id8_v)
            awT = sb.tile([W, BC], f32)
            nc.vector.tensor_copy(awT[:], awT_ps[:])

            # ---- context[b, h] = sum_w aw[b, w] * enc[w, b, h]
            # ([1, BC*H] PSUM row == row-major [BC, H] when flattened)
            ctx_ps = ps.tile([1, BC * H], f32)
            for b in range(BC):
                nc.tensor.matmul(out=ctx_ps[:, b * H:(b + 1) * H],
                                 lhsT=awT[:, b:b + 1],
                                 rhs=enc_sb[:, b, :],
                                 start=True, stop=True)
            ctx_sb = sb.tile([1, BC * H], f32)
            nc.vector.tensor_copy(ctx_sb[:, 0:512], ctx_ps[:, 0:512])
            nc.scalar.copy(out=ctx_sb[:, 512:1024], in_=ctx_ps[:, 512:1024])
            nc.sync.dma_start(
                ctx_o[:].rearrange("b h -> (b h)")[None, :], ctx_sb[:])

            # ---- scatter windows into the bulk outputs
            nc.gpsimd.indirect_dma_start(
                out=alif_o[:],
                out_offset=bass.IndirectOffsetOnAxis(ap=widx_v, axis=1),
                in_=aw[:], in_offset=None)
            nc.gpsimd.indirect_dma_start(
                out=cumn_o[:],
                out_offset=bass.IndirectOffsetOnAxis(ap=widx_v, axis=1),
                in_=aw[:], in_offset=None, compute_op=OP.add)

            # ---- ws_new = clip(ws + argmax(aw) - W//2, 0, ntok - W)
            amax = sb.tile([BC, 1], f32)
            nc.vector.reduce_max(amax[:], aw[:], axis=AX)
            eqm = sb.tile([BC, W], f32)
            nc.vector.tensor_scalar(out=eqm[:], in0=aw[:], scalar1=amax[:],
                                    scalar2=None, op0=OP.is_equal)
            t1 = sb.tile([BC, W], f32)
            nc.vector.tensor_scalar(out=t1[:], in0=eqm[:], scalar1=-4096.0,
                                    scalar2=4096.0, op0=OP.mult, op1=OP.add)
            t2 = sb.tile([BC, W], f32)
            nc.vector.tensor_mul(t2[:], iot_v, eqm[:])
            idxm = sb.tile([BC, W], f32)
            nc.vector.tensor_add(idxm[:], t1[:], t2[:])
            fidx = sb.tile([BC, 1], f32)
            nc.vector.tensor_reduce(fidx[:], idxm[:], axis=AX, op=OP.min)
            wsn = sb.tile([BC, 1], f32)
            nc.vector.tensor_scalar(out=wsn[:], in0=fidx[:],
                                    scalar1=wsf_v, scalar2=float(-(W // 2)),
                                    op0=OP.add, op1=OP.add)
            ntf = sb.tile([BC, 1], f32)
            nc.vector.tensor_copy(ntf[:], ntok_v)
            lim = sb.tile([BC, 1], f32)
            nc.vector.tensor_scalar_add(lim[:], ntf[:], float(-W))
            wsn2 = sb.tile([BC, 1], f32)
            nc.vector.tensor_tensor(wsn2[:], wsn[:], lim[:], op=OP.min)
            wsn3 = sb.tile([BC, 1], f32)
            nc.vector.tensor_scalar_max(wsn3[:], wsn2[:], 0.0)
            wsn_i = sb.tile([BC, 1], i32)
            nc.vector.tensor_copy(wsn_i[:], wsn3[:])
            nc.sync.dma_start(wsn_o[:], wsn_i[:])

    nc.compile()
    return nc


def _prep_core_inputs(core, encoded_tokens, tokens_mask, num_tokens, query,
                      cumulative_alignment, initial_cumulative_alignment,
                      window_start, wq_l, cwt_l, v_col, bhb_col):
    bs = slice(core * BC, (core + 1) * BC)
    ws = window_start[bs].astype(np.int64)

    qc = query[0, bs, :]                       # [BC, Q]
    q_l = np.ascontiguousarray(
        qc.T.reshape(8, 128, BC).transpose(1, 0, 2).reshape(128, 8 * BC))

    # dma_gather indices: unwrapped L[i] (i = b*W + w) = row b*T + ws_b + w
    # of enc viewed as [(b t), h]; wrapped layout [p, s] = L[s*16 + p%16],
    # replicated across the 8 GPSIMD cores (partition groups of 16).
    w_ar = np.arange(W, dtype=np.int64)
    L = (np.arange(BC, dtype=np.int64)[:, None] * T
         + ws[:, None] + w_ar[None, :]).reshape(-1)             # [BC*W]
    wrapped = L.reshape((BC * W) // 16, 16).T                   # [16, n/16]
    gidx = np.tile(wrapped, (8, 1)).astype(np.int16)            # [128, n/16]

    # packed [BC, S8C] i32 tensor
    s8 = np.zeros((BC, S8C), dtype=np.int32)
    s8[:, 0] = (np.arange(BC) * LP + ws).astype(np.int32)       # lidx
    s8[:, 1] = (np.arange(BC) * T + ws).astype(np.int32)        # widx
    s8[:, 2] = num_tokens[bs].astype(np.int32)                  # ntok
    s8[:, 3] = ws.astype(np.float32).view(np.int32)             # wsf
    s8[:, 4] = initial_cumulative_alignment[bs, 0].astype(
        np.float32).view(np.int32)                              # initv
    s8[:, 8:16] = np.eye(BC, dtype=np.float32).view(np.int32)   # id8
    s8[:, 16:144] = np.tile(np.arange(W, dtype=np.float32),
                            (BC, 1)).view(np.int32)             # iota

    # packed [128, S128C] i32 tensor
    s128 = np.zeros((128, S128C), dtype=np.int32)
    s128[:, 0:32] = gidx.view(np.int32)
    s128[:, 32:96] = q_l.astype(np.float32).view(np.int32)
    s128[:, 96] = v_col[:, 0].view(np.int32)
    s128[:, 97] = bhb_col[:, 0].view(np.int32)
    s128[0:31, 98:226] = cwt_l.view(np.int32)
    s128[:, 226] = np.float32(1.0).view(np.int32)

    return {
        "enc": np.ascontiguousarray(encoded_tokens[:, bs, :].transpose(1, 0, 2)),
        "mask": np.ascontiguousarray(tokens_mask[bs, :]).astype(np.uint8),
        "cum": np.ascontiguousarray(cumulative_alignment[bs, :]),
        "wq": wq_l,
        "sm8": s8,
        "sm128": s128,
    }


def make_in_maps(encoded_tokens, tokens_mask, num_tokens, query,
                 cumulative_alignment, initial_cumulative_alignment,
                 window_start, Wq, bq, conv_w, conv_b, v):
    encoded_tokens = np.asarray(encoded_tokens, dtype=np.float32)
    tokens_mask = np.asarray(tokens_mask)
    num_tokens = np.asarray(num_tokens, dtype=np.int32)
    query = np.asarray(query, dtype=np.float32)
    cumulative_alignment = np.asarray(cumulative_alignment, dtype=np.float32)
    initial_cumulative_alignment = np.asarray(
        initial_cumulative_alignment, dtype=np.float32)
    window_start = np.asarray(window_start, dtype=np.int32)
    Wq = np.asarray(Wq, dtype=np.float32)
    bq = np.asarray(bq, dtype=np.float32)
    conv_w = np.asarray(conv_w, dtype=np.float32)
    conv_b = np.asarray(conv_b, dtype=np.float32)
    v = np.asarray(v, dtype=np.float32)

    wq_l = np.ascontiguousarray(
        Wq.T.reshape(8, 128, H).transpose(1, 0, 2).reshape(128, Q))
    cwt_l = np.ascontiguousarray(conv_w[:, 0, :].T)     # [K, H]
    v_col = np.ascontiguousarray(v.reshape(H, 1))
    bhb_col = np.ascontiguousarray((bq + conv_b).reshape(H, 1))

    return [
        _prep_core_inputs(c, encoded_tokens, tokens_mask, num_tokens, query,
                          cumulative_alignment, initial_cumulative_alignment,
                          window_start, wq_l, cwt_l, v_col, bhb_col)
        for c in range(NCORES)
    ]


def kernel(**inputs):
    global last_results
    trace = bool(os.environ.get("KERNEL_TRACE"))
    if trace:
        _install_trace_hook()

    from concourse.bass_utils import run_bass_kernel_spmd

    if "nc" not in _CACHE:
        _CACHE["nc"] = _build_program()
    nc = _CACHE["nc"]

    in_maps = make_in_maps(**inputs)
    res = run_bass_kernel_spmd(nc, in_maps, core_ids=list(range(NCORES)),
                               trace=trace)
    last_results = res

    context = np.concatenate([r["context"] for r in res.results], axis=0)
    cum_new = np.concatenate([r["cum_new"] for r in res.results], axis=0)
    align_full = np.concatenate([r["align_full"] for r in res.results], axis=0)
    ws_new = np.concatenate(
        [r["ws_new"].reshape(BC) for r in res.results], axis=0)
    return context, cum_new, align_full, ws_new
